# revision 25
# baseline (speedup 1.0000x reference)
"""BERT-style self-attention for Trainium2, data-parallel over batch (8 cores).

Problem: B=8, S=512, H=768, NH=12, HD=64.
Each core handles one batch element end-to-end (no collectives):
  q = h @ Wq.T + bq ; k = h @ Wk.T + bk ; v = h @ Wv.T
  scores = q k^T / 8 ; probs = softmax(scores + mask) ; ctx = probs (v + bv)

Design (v2):
- Transposed scores (k on partitions, q free) so softmax needs no transposes:
  E[k,q] = exp(scoresT/8); mask is folded into v as v' = v * exp(mask[k])
  (exact: ctx = sum_k E*exp(mask)*v / sum_k E*exp(mask)), which makes the
  exp bias-free/constant-scale so score tiles fuse into [128,1024] ACTIVATEs
  (the ACT engine is the attention-phase co-bottleneck).
- bv is added on the host (exact: sum_k p_k = 1 -> ctx(v+bv) = ctx(v)+bv).
- ctxT_ext = [v'|exp(mask)]^T @ E gives unnormalized ctx AND Z in one matmul;
  PE-transpose back per q-tile, normalize by 1/Z on DVE.
- Head-PAIR structure: the two heads of a pair live at partition offsets 0/64
  of one 128-feature chunk, so their K=64 score matmuls run concurrently in
  disjoint PE row groups.
- Fully interleaved schedule: per pair hp: q[hp],k[hp],scores[hp],v[hp], then
  ctx/transpose/normalize/out-DMA of pair hp-1.  Keeps the PE busy end to end
  (HAM un-throttles after ~3.4us of sustained activity and re-throttles after
  ~3.4us idle - avoid gaps), starts the ACT exp stream ~30us earlier than a
  phase-ordered kernel, and drains outputs continuously (no DMA tail).
- DMA ordering: hT+wq interleaved on the sync+gpsimd queues first (q needs
  them first), wk issued on scalar gated behind wq's last chunk, wv issued on
  gpsimd gated behind wk's last chunk: each phase gets the full ~300GB/s.
- PSUM evictions split across engines: q/k->DVE (bias add), v->Pool
  (exp(mask) scale), ctxT->Pool (copy), normalize on DVE.
"""

import os
import sys

for _p in ("/opt/trn_rl_repo", "/root/.axon_site/_ro/trn_rl_repo"):
    if os.path.isdir(_p) and _p not in sys.path:
        sys.path.insert(0, _p)

import numpy as np

from concourse import bacc, bass, tile
import concourse.mybir as mybir
from concourse.bass_utils import run_bass_kernel_spmd
from concourse.masks import make_identity

B, S, H, NH = 8, 512, 768, 12
HD = H // NH        # 64
P = 128
NC_ = H // P        # 6 feature chunks of 128 (== head pairs)
NS = S // P         # 4 sequence tiles of 128
NP_ = NH // 2       # 6 head pairs
HE = HD + 1         # 65: head dim + Z column
F32 = mybir.dt.float32
F16 = mybir.dt.float16
NP_IN = np.float16

N_WARMUP = int(os.environ.get("KERNEL_WARMUP", "48"))


def build_nc():
    nc = bacc.Bacc(None, target_bir_lowering=False, debug=False)

    # ---- DRAM parameters (per-core views prepared on host) ----
    hT = nc.declare_dram_parameter("hT", [H, S], F16, isOutput=False)
    wqT = nc.declare_dram_parameter("wqT", [H, H], F16, isOutput=False)
    wkT = nc.declare_dram_parameter("wkT", [H, H], F16, isOutput=False)
    wvT = nc.declare_dram_parameter("wvT", [H, H], F16, isOutput=False)
    bq = nc.declare_dram_parameter("bq_pt", [P, NC_], F32, isOutput=False)
    bk = nc.declare_dram_parameter("bk_pt", [P, NC_], F32, isOutput=False)
    mask = nc.declare_dram_parameter("mask_pt", [P, NS], F32, isOutput=False)
    out = nc.declare_dram_parameter("out", [S, H], F32, isOutput=True)

    with tile.TileContext(nc) as tc:
        with (
            tc.tile_pool(name="consts", bufs=1) as consts,
            tc.tile_pool(name="inp", bufs=1) as inp,
            tc.tile_pool(name="qk", bufs=2) as qkp,
            tc.tile_pool(name="epool", bufs=16) as epool,
            tc.tile_pool(name="csbp", bufs=4) as csbp,
            tc.tile_pool(name="outp", bufs=1) as outp,
            tc.tile_pool(name="rpool", bufs=8) as rpool,
            tc.tile_pool(name="gatep", bufs=1) as gatep,
            tc.tile_pool(name="ps_a", bufs=2, space="PSUM") as ps_a,
            tc.tile_pool(name="ps_sc", bufs=4, space="PSUM") as ps_sc,
            tc.tile_pool(name="ps_ctx", bufs=2, space="PSUM") as ps_ctx,
        ):
            # ---- input DMAs. Two constraints drive the layout:
            # (1) each DMA_DIRECT2D *issue* costs ~700ns serially on its
            #     engine, so use few, 2-chunk sub-DMAs (12 total, 4/queue);
            # (2) one queue tops out ~160GB/s, so run all three queues in
            #     parallel, round-robining sub-DMAs in PE-consumption
            #     priority order (hT+wq interleaved, then wk, then wv). ----
            hT_sb = inp.tile([P, NC_, S], F16)
            wq_sb = inp.tile([P, NC_, H], F16)
            wk_sb = inp.tile([P, NC_, H], F16)
            wv_sb = inp.tile([P, NC_, H], F16)
            order = []
            for c in range(0, NC_, 2):
                order.append((hT_sb[:, c:c + 2, :], hT, c))
                order.append((wq_sb[:, c:c + 2, :], wqT, c))
            for c in range(0, NC_, 2):
                order.append((wk_sb[:, c:c + 2, :], wkT, c))
            for c in range(0, NC_, 2):
                order.append((wv_sb[:, c:c + 2, :], wvT, c))
            queues = (nc.sync, nc.scalar, nc.gpsimd)
            for i, (dst, src, c) in enumerate(order):
                queues[i % 3].dma_start(
                    out=dst,
                    in_=src[c * P:(c + 2) * P, :].rearrange(
                        "(c p) s -> p c s", p=P))

            # ---- const DMAs (sync queue, after the critical input issues)
            mask_sb = consts.tile([P, NS], F32)
            nc.sync.dma_start(out=mask_sb[:], in_=mask[:])
            bq_sb = consts.tile([P, NC_], F32)
            nc.sync.dma_start(out=bq_sb[:], in_=bq[:])
            bk_sb = consts.tile([P, NC_], F32)
            nc.sync.dma_start(out=bk_sb[:], in_=bk[:])

            # identity for PE transposes (gpsimd iota; after the DMA issues)
            ident = consts.tile([P, P], F16)
            make_identity(nc, ident)

            # exp(mask): per-k-position softmax mask factor, folded into v'
            expm = consts.tile([P, NS], F32)
            nc.scalar.activation(
                expm[:], mask_sb[:], mybir.ActivationFunctionType.Exp)

            # v_ext[:, st, 65h+d] = v'[st*128+p, 64h+d]; v_ext[:, st, 65h+64]
            # = exp(mask). Ones-columns written once via Pool.
            v_ext = qkp.tile([P, NS, NH * HE], F16, tag="vext")
            ones12 = consts.tile([P, NH], F16)
            nc.vector.memset(ones12[:], 1.0)
            for st in range(NS):
                zcols = v_ext[:, st, :].rearrange(
                    "p (h e) -> p h e", e=HE)[:, :, HD]
                nc.gpsimd.tensor_scalar_mul(
                    out=zcols, in0=ones12[:], scalar1=expm[:, st:st + 1])

            # ---- PE warm-up: HAM un-throttles (1.2 -> 2.4 GHz) only after
            # ~3.4us sustained activity, and the input DMAs take ~7us after
            # the first packets; bridge the whole window so the PE enters the
            # real work warm and never idles >3.4us. ----
            warm_in = consts.tile([P, P], F16)
            nc.vector.memset(warm_in[:], 1.0)
            warm_ps = ps_sc.tile([P, P], F32, tag="sc")

            def emit_warmup(n):
                for _ in range(n):
                    nc.tensor.matmul(warm_ps[:], warm_in[:], warm_in[:],
                                     start=True, stop=True)

            emit_warmup(N_WARMUP)

            out_sb = outp.tile([P, NS, H], F32)
            SCALE = 1.0 / np.sqrt(np.float32(HD))

            def emit_qk(hp, warm_between=0):
                outs = []
                for w_sb, b_sb, tag in ((wq_sb, bq_sb, "qT"),
                                        (wk_sb, bk_sb, "kT")):
                    ps = ps_a.tile([P, S], F32, tag="a")
                    for ic in range(NC_):
                        nc.tensor.matmul(
                            ps[:],
                            w_sb[:, ic, hp * P:(hp + 1) * P],
                            hT_sb[:, ic, :],
                            start=(ic == 0), stop=(ic == NC_ - 1),
                        )
                    dst = qkp.tile([P, S], F16, tag=tag)
                    nc.vector.tensor_scalar_add(
                        out=dst[:], in0=ps[:], scalar1=b_sb[:, hp:hp + 1])
                    outs.append(dst)
                    # iteration 0 is input-DMA paced: keep the PE busy with
                    # warm-up matmuls so HAM never sees an idle window
                    emit_warmup(warm_between)
                return outs

            def emit_scores(hp, qT, kT):
                # E[j][kt]: fp16 exp tiles [128, 512], one per (head, kt).
                # The two heads' matmuls at row offsets 0/64 run concurrently;
                # 4 independent sc psum banks decouple the PE from ACT's exp
                # latency (a buffer's exp is done before the PE rotates back).
                es = [[], []]
                for kt in range(NS):
                    for j in range(2):
                        sp = ps_sc.tile([P, S], F32, tag="sc", name="sc_ps")
                        off = j * HD
                        nc.tensor.matmul(
                            sp[:],
                            kT[off:off + HD, kt * P:(kt + 1) * P],
                            qT[off:off + HD, :],
                            start=True, stop=True,
                        )
                        e = epool.tile([P, S], F16, tag="e")
                        nc.scalar.activation(
                            e[:], sp[:],
                            mybir.ActivationFunctionType.Exp, scale=SCALE)
                        es[j].append(e)
                return es

            def emit_v(hp):
                for st in range(NS):
                    # v psum rides the ps_sc rotation so ps_a stays a pure
                    # q/k ping-pong (q of iter i must not wait on iter i-1's
                    # late v evictions)
                    ps = ps_sc.tile([P, P], F32, tag="sc", name="v_ps")
                    for ic in range(NC_):
                        nc.tensor.matmul(
                            ps[:],
                            hT_sb[:, ic, st * P:(st + 1) * P],
                            wv_sb[:, ic, hp * P:(hp + 1) * P],
                            start=(ic == 0), stop=(ic == NC_ - 1),
                        )
                    dst = v_ext[:, st, hp * 2 * HE:(hp + 1) * 2 * HE].rearrange(
                        "p (h e) -> p h e", e=HE)[:, :, 0:HD]
                    # v' = v * exp(mask): fold the softmax mask into v during
                    # the DVE eviction (Pool can't read PSUM; ACT is busy
                    # with the exp stream)
                    nc.vector.tensor_scalar_mul(
                        out=dst,
                        in0=ps[:].rearrange("p (h d) -> p h d", d=HD),
                        scalar1=expm[:, st:st + 1])

            def emit_ctx(h, e_tiles):
                # ctxT_ext [65, 512]: rows 0..63 = v'^T E, row 64 = Z
                cps = ps_ctx.tile([HE, S], F32, tag="ctx")
                for kt in range(NS):
                    nc.tensor.matmul(
                        cps[:],
                        v_ext[:, kt, h * HE:(h + 1) * HE],
                        e_tiles[kt][:],
                        start=(kt == 0), stop=(kt == NS - 1),
                    )
                csb = csbp.tile([HE, S], F16, tag="csb")
                if h % 2 == 0:  # balance the two evictions across ACT/DVE
                    nc.scalar.activation(
                        csb[:], cps[:], mybir.ActivationFunctionType.Copy)
                else:
                    nc.vector.tensor_copy(out=csb[:], in_=cps[:])
                return csb

            def emit_tail(hp, csb0, csb1):
                # Per qt: PE-transpose both heads' ctxT back to [q, d] into
                # one small fp16 PSUM tile, evict it immediately with a fast
                # DVE cast (keeps the PE's psum rotation unblocked), then
                # normalize by 1/Z on the otherwise-idle Pool engine.
                TW = HE + 1  # 66: pad so the 2nd slot is 4-byte aligned
                for qt in range(NS):
                    tp = ps_ctx.tile([P, 2 * TW], F16, tag="ctx", name="tp")
                    nc.tensor.transpose(
                        tp[:, 0:HE], csb0[:, qt * P:(qt + 1) * P],
                        ident[0:HE, 0:HE])
                    nc.tensor.transpose(
                        tp[:, TW:TW + HE], csb1[:, qt * P:(qt + 1) * P],
                        ident[0:HE, 0:HE])
                    tps = rpool.tile([P, 2 * TW], F32, tag="rp", name="tps")
                    nc.vector.tensor_copy(out=tps[:], in_=tp[:])
                    for j in range(2):
                        h = 2 * hp + j
                        nc.gpsimd.normalize_recip(
                            out_sb[:, qt, h * HD:(h + 1) * HD],
                            tps[:, j * TW:j * TW + HD],
                            tps[:, j * TW + HD:j * TW + HE])
                eng = (nc.sync, nc.gpsimd, nc.scalar)[hp % 3]
                eng.dma_start(
                    out=out[:, hp * P:(hp + 1) * P].rearrange(
                        "(t p) c -> p t c", p=P),
                    in_=out_sb[:, :, hp * P:(hp + 1) * P])

            # Software pipeline: iteration hp runs q,k of pair hp, then pair
            # hp-1's ctx + transposes (fills the PE gap while DVE evicts k,
            # and keeps the DVE tp-evictions early in its FIFO so nothing
            # downstream head-of-line blocks), then scores and v of hp.
            prev = None
            for hp in range(NP_):
                qT, kT = emit_qk(hp, warm_between=14 if hp == 0 else 0)
                if prev is not None:
                    php, pes = prev
                    csb0 = emit_ctx(2 * php, pes[0])
                    csb1 = emit_ctx(2 * php + 1, pes[1])
                    emit_tail(php, csb0, csb1)
                es = emit_scores(hp, qT, kT)
                if hp == 0:
                    emit_warmup(10)
                emit_v(hp)
                prev = (hp, es)
            php, pes = prev
            csb0 = emit_ctx(2 * php, pes[0])
            csb1 = emit_ctx(2 * php + 1, pes[1])
            emit_tail(php, csb0, csb1)

    nc.compile()
    return nc


def _prep_inputs(hidden_states, attention_mask, Wq, bq, Wk, bk, Wv, bv):
    """Host-side shard + layout prep. Returns per-core input maps."""
    f32 = np.float32
    wqT = np.ascontiguousarray(Wq.T, dtype=NP_IN)
    wkT = np.ascontiguousarray(Wk.T, dtype=NP_IN)
    wvT = np.ascontiguousarray(Wv.T, dtype=NP_IN)
    bq_pt = np.ascontiguousarray(bq.reshape(NC_, P).T, dtype=f32)
    bk_pt = np.ascontiguousarray(bk.reshape(NC_, P).T, dtype=f32)
    in_maps = []
    for b in range(B):
        mask_pt = np.ascontiguousarray(
            attention_mask[b, 0, 0, :].reshape(NS, P).T, dtype=f32)
        in_maps.append({
            "hT": np.ascontiguousarray(hidden_states[b].T, dtype=NP_IN),
            "wqT": wqT, "wkT": wkT, "wvT": wvT,
            "bq_pt": bq_pt, "bk_pt": bk_pt, "mask_pt": mask_pt,
        })
    return in_maps


_NC_CACHE = None


def _install_ntff_hook():
    """Provide antenv.axon_hooks.get_axon_ntff_profile_hook via ctypes on
    libaxon_pjrt.so (the image's antenv stub lacks the submodule)."""
    import contextlib
    import ctypes
    import types

    try:
        import antenv.axon_hooks  # noqa: F401
        return True
    except ImportError:
        pass
    so_path = "/opt/axon/libaxon_pjrt.so"
    if not os.path.exists(so_path):
        return False
    lib = ctypes.CDLL(so_path)
    if not hasattr(lib, "axon_start_nrt_profile"):
        return False
    lib.axon_start_nrt_profile.argtypes = [
        ctypes.POINTER(ctypes.c_int64), ctypes.c_size_t]
    lib.axon_start_nrt_profile.restype = ctypes.c_int64
    lib.axon_stop_nrt_profile.argtypes = [ctypes.c_char_p]
    lib.axon_stop_nrt_profile.restype = ctypes.c_int64

    @contextlib.contextmanager
    def _hook(output_dir, device_ids):
        import jax
        jax.devices()
        if device_ids:
            ids = (ctypes.c_int64 * len(device_ids))(*device_ids)
            rc = lib.axon_start_nrt_profile(ids, len(device_ids))
        else:
            rc = lib.axon_start_nrt_profile(None, 0)
        if rc != 0:
            raise RuntimeError(f"axon_start_nrt_profile rc={rc}")
        try:
            yield
        finally:
            n = lib.axon_stop_nrt_profile(str(output_dir).encode())
            print(f"ntff profile: {n} file(s) -> {output_dir}", file=sys.stderr)

    import antenv
    mod = types.ModuleType("antenv.axon_hooks")
    mod.get_axon_ntff_profile_hook = lambda: _hook
    mod.set_axon_ntff_profile_hook = lambda h: None
    sys.modules["antenv.axon_hooks"] = mod
    antenv.axon_hooks = mod
    return True


def run(trace=False, tmpdir=None, **inputs):
    global _NC_CACHE
    if _NC_CACHE is None:
        _NC_CACHE = build_nc()
    if trace:
        trace = _install_ntff_hook()
    bv = np.asarray(inputs["bv"], np.float32)
    in_maps = _prep_inputs(**inputs)
    res = run_bass_kernel_spmd(
        _NC_CACHE, in_maps, list(range(B)), trace=trace, tmpdir=tmpdir)
    out = np.stack([res.results[b]["out"] for b in range(B)], axis=0)
    out += bv  # exact: sum_k p_k = 1, so ctx(v + bv) = ctx(v) + bv
    return out, res


def kernel(**inputs):
    out, _ = run(trace=False, **inputs)
    return out


if __name__ == "__main__":
    rng = np.random.default_rng(0)
    hs = rng.standard_normal((B, S, H)).astype(np.float32)
    am = np.zeros((B, 1, 1, S), np.float32)
    mk = lambda: (rng.standard_normal((H, H)).astype(np.float32) * 0.02)
    o = kernel(hidden_states=hs, attention_mask=am,
               Wq=mk(), bq=np.zeros(H, np.float32),
               Wk=mk(), bk=np.zeros(H, np.float32),
               Wv=mk(), bv=np.zeros(H, np.float32))
    print(o.shape, o.dtype)
